# revision 35
# baseline (speedup 1.0000x reference)
"""BDGCN (dual-diffusion graph conv) Trainium2 kernel.

Math (per batch b):
  m1[k,m,c,l] = sum_n X[n,c,l] G[k,n,m]
  m2[m,d,k,j,l] = sum_c m1[k,m,c,l] G[j,c,d]
  out[m,d,h] = relu(sum_{k,j,l} m2[m,d,k,j,l] W[k*96+j*32+l, h] + b[h])

Sharding: data-parallel over batch; B=8 -> one batch per NeuronCore,
G/W/b replicated. No collectives.

Per-core pipeline, all-bf16 operands with fp32 PSUM accumulation:
  Phase 1 (contract n): stationary = X[n128, c128 @ l] bf16,
           moving = G (k-paired) [n128, (k2,m256)=512] -> psum
           [c128, (k,m)], accum over 2 n-chunks; one DVE/Pool/Act copy
           per (cc,l) into m1[cc] bf16, free layout (k,g,l,r), m=4g+r.
  Phase 2 (contract c): stationary = m1[cc][k,g] (128 cols = (l,r)),
           moving = G (j-paired) [c128, (j2,d256)] -> psum
           [(l,r)128, (j,d)], accum over 2 c-chunks -> m2 SBUF bf16
           tiles [128, 768] per (g,k).
  Phase 3 (contract (k,j,l)): stationary = m2[g,k][:, j,dc-slice],
           moving = block-diagonal W [(l,r)128, (r,h)256] -> psum
           [d128, (m4,h)256], accum over 9 (k,j). Then +bias (DVE
           scalar_tensor_tensor), relu (Pool), DMA out to [m, d, h].
  Phase 3 of group g is emitted after phase 2 of group g+1 so the PE
  never waits on the psum->SBUF copies.

Walrus-build workarounds baked in: Tile's exit drain is split into
single-wait drains (_patch_tile_drain) and any instruction carrying >1
semaphore wait gets extra waits hoisted onto NoOps (_split_multi_waits).
"""

import numpy as np

B, N, L, K, H = 8, 256, 32, 3, 64
P = 128  # partitions

_CACHE = {}


def _patch_tile_drain():
    """This container's walrus build rejects instructions carrying more
    than one semaphore wait; Tile's exit emits one drain with N waits.
    Split it into N single-wait drains."""
    import concourse.mybir as mybir
    import concourse.tile as tile

    if getattr(tile.TileContext, "_drain_split_patched", False):
        return

    def patched(self, tick_clock, wait_clock):
        from concourse.vector_clock import ScopedClock

        nc = self.nc
        probe = nc.sync.drain()
        wait_clock.add_sem_waits(
            probe.ins, ScopedClock({None: tick_clock.global_clock})
        )
        si = probe.ins.sync_info
        waits = list(si.on_wait) if si is not None else []
        if len(waits) > 1:
            si.on_wait = [waits[0]]
            for w in waits[1:]:
                d = nc.sync.drain()
                d.ins.sync_info = mybir.SyncInfo(on_update=[], on_wait=[w])
        nc.all_engine_barrier()
        assert self.sems is not None
        popped = nc._tile_sem_poison_stack.pop()
        assert popped is self._sem_poison
        nc.clear_and_free_semaphores(list(self.sems.allocated().values()))
        nc.all_engine_barrier()

    tile.TileContext._drain_and_barrier = patched
    tile.TileContext._drain_split_patched = True


def _build_nc(reps=1, mode="full", split_waits=True):
    import concourse.bass as bass
    import concourse.mybir as mybir
    import concourse.tile as tile

    _patch_tile_drain()

    f32 = mybir.dt.float32
    bf16 = mybir.dt.bfloat16
    nc = bass.Bass("TRN2", target_bir_lowering=False, debug=False)

    Xd = nc.dram_tensor("X", [N, N, L], bf16, kind="ExternalInput")
    GBd = nc.dram_tensor("GB", [K, N, N], bf16, kind="ExternalInput")
    Wd = nc.dram_tensor("WT", [P, K * H], bf16, kind="ExternalInput")
    Bd = nc.dram_tensor("BR", [1, 8 * H], bf16, kind="ExternalInput")
    Od = nc.dram_tensor("OUT", [N, N, H], f32, kind="ExternalOutput")

    NC2 = N // P   # 2 chunks of 128 along n / c / d
    MG = 4         # m's per group
    NG = N // MG   # 64 groups over all m
    KLP = P        # (k,l) contraction padded 96 -> 128
    JH = K * H     # 192 = (j, h) moving width in T-build

    cp = mybir.ActivationFunctionType.Copy
    relu = mybir.ActivationFunctionType.Relu

    with tile.TileContext(nc) as tc:
        with (
            tc.tile_pool(name="big", bufs=1) as big,
            tc.tile_pool(name="m1tp", bufs=12) as m1tp,
            tc.tile_pool(name="tsp", bufs=10) as tsp,
            tc.tile_pool(name="outp", bufs=8) as outp,
            tc.tile_pool(name="tp", bufs=6, space="PSUM") as tpp,
            tc.tile_pool(name="ps3", bufs=2, space="PSUM") as ps3p,
        ):
            # ---- resident loads ----
            xsb = big.tile([P, NC2 * N * L], bf16, tag="xsb")
            x4 = xsb.rearrange("p (b c l) -> p b c l", b=NC2, c=N)
            nc.sync.dma_start(
                out=x4, in_=Xd[:, :, :].rearrange("(b p) c l -> p b c l", p=P)
            )
            gsb = big.tile([P, NC2 * K * N], bf16, tag="gsb")
            g4 = gsb.rearrange("p (b k m) -> p b k m", b=NC2, k=K)
            for k in range(K):
                nc.sync.dma_start(
                    out=g4[:, :, k, :],
                    in_=GBd[k, :, :].rearrange("(b p) m -> p b m", p=P),
                )
            wkl = big.tile([P, JH], bf16, tag="wkl")
            nc.sync.dma_start(out=wkl, in_=Wd[:, :])
            brow = big.tile([1, 8 * H], bf16, tag="brow")
            nc.sync.dma_start(out=brow, in_=Bd[:, :])
            ones = big.tile([1, P], bf16, tag="ones")
            nc.gpsimd.memset(ones[:, :], 1.0)

            # m1[cc]: [c 128, (g 64, r 4, klp 128)] bf16; klp = k*32+l, padded
            m1 = {}
            for cc in range(NC2):
                t = big.tile(
                    [P, NG * MG * KLP], bf16,
                    tag=f"m1_{cc}", name=f"m1_{cc}",
                )
                v = t.rearrange(
                    "p (g r k l) -> p g r k l", g=NG, r=MG, k=MG
                )  # k-dim 4 = 3 real + 1 pad block of 32
                # zero the pad block (k==3) once
                nc.gpsimd.memset(v[:, :, :, K, :], 0.0)
                if mode in ("pe_only", "dma_out"):
                    nc.gpsimd.memset(t[:, :], 0.25)
                m1[cc] = v

            static_ost = None
            if mode == "dma_out":
                static_ost = big.tile([P, 512], f32, tag="sost",
                                      name="sost")
                nc.gpsimd.memset(static_ost[:, :], 1.0)
            static_ts = None
            if mode in ("pe_only", "dma_out"):
                static_ts = big.tile([P, MG * JH], bf16, tag="sts",
                                     name="sts")
                nc.gpsimd.memset(static_ts[:, :], 0.25)

            cp_state = [0]

            def copy_on(out, in_):
                # psum -> SBUF; only DVE and Act can read PSUM
                if mode in ("pe_only", "dma_out"):
                    return
                e = cp_state[0] % 2
                cp_state[0] += 1
                if e == 0:
                    nc.vector.tensor_copy(out, in_)
                else:
                    nc.scalar.activation(out, in_, cp)

            def phase1():
                for cc in range(NC2):
                    bpend = None
                    for l in range(L):
                        pa = tpp.tile([P, 512], f32, tag="tp", name="pa")
                        if bpend is None:
                            bpend = tpp.tile([P, 512], f32, tag="tp",
                                             name="pb")
                            boff = 0
                        else:
                            boff = N
                        for nchk in range(NC2):
                            st = x4[:, nchk, cc * P:(cc + 1) * P, l]
                            nc.tensor.matmul(
                                pa, lhsT=st, rhs=g4[:, nchk, 0:2, :],
                                start=(nchk == 0), stop=(nchk == 1),
                                skip_group_check=True,
                            )
                            nc.tensor.matmul(
                                bpend[:, boff:boff + N], lhsT=st,
                                rhs=g4[:, nchk, 2, :],
                                start=(nchk == 0), stop=(nchk == 1),
                                skip_group_check=True,
                            )
                        # psum cols (k2, m), m = 4g+r -> m1[cc][g, r, k, l]
                        copy_on(
                            m1[cc][:, :, :, 0:2, l],
                            pa.rearrange("p (k g r) -> p g r k", k=2, g=NG),
                        )
                        if boff == N:
                            copy_on(
                                m1[cc][:, :, :, 2, l - 1:l + 1],
                                bpend.rearrange(
                                    "p (h g r) -> p g r h", h=2, g=NG
                                ),
                            )
                            bpend = None

            # ---- transpose + T-build + final, pipelined over g ----
            m1flat = {cc: m1[cc].rearrange("p g r k l -> p (g r k l)")
                      for cc in range(NC2)}
            m1t = {}

            static_m1t = None
            if mode == "no_trans":
                static_m1t = big.tile([P, MG * P], bf16, tag="sm1t",
                                      name="sm1t")
                nc.gpsimd.memset(static_m1t[:, :], 0.25)

            def trans(g):
                for cc in range(NC2):
                    if mode == "no_trans":
                        m1t[cc, g] = static_m1t
                        continue
                    t = m1tp.tile([P, MG * P], bf16, tag="m1t", name="m1t")
                    eng = nc.scalar if cc == 0 else nc.sync
                    eng.dma_start_transpose(
                        t.rearrange("p (r c) -> p r c", r=MG),
                        m1flat[cc][:, g * MG * KLP:(g + 1) * MG * KLP],
                    )
                    m1t[cc, g] = t

            def tbuild(g):
                tss = []
                for cc in range(NC2):
                    mt = m1t[cc, g].rearrange(
                        "p (r c) -> p r c", r=MG
                    )
                    if mode in ("pe_only", "dma_out"):
                        ts = static_ts
                    else:
                        ts = tsp.tile([P, MG * JH], bf16, tag="ts",
                                      name="ts")
                    for rr in range(2):  # r-pairs
                        pt = tpp.tile([P, 512], f32, tag="tp", name="pt")
                        for ri in range(2):
                            r = rr * 2 + ri
                            nc.tensor.matmul(
                                pt[:, ri * JH:(ri + 1) * JH],
                                lhsT=mt[:, r, :], rhs=wkl[:, :],
                                start=True, stop=True,
                                skip_group_check=True,
                            )
                        copy_on(ts[:, rr * 2 * JH:(rr + 1) * 2 * JH],
                                pt[:, 0:2 * JH])
                    tss.append(ts)
                return tss

            def p3_start(ps3):
                nc.tensor.matmul(
                    ps3, lhsT=ones, rhs=brow,
                    start=True, stop=False, skip_group_check=True,
                )

            def final(g, tss, ps3):
                # out[m,d,h] += sum_c G_j[c,d] * T[c, (r,j,h)]
                n_mm = NC2 * K * NC2
                i = 0
                for dc in range(NC2):
                    for j in range(K):
                        for cc in range(NC2):
                            i += 1
                            ts = tss[cc]
                            tv = ts.rearrange(
                                "p (r j h) -> p r j h", r=MG, j=K
                            )
                            nc.tensor.matmul(
                                ps3[:, dc * N:(dc + 1) * N],
                                lhsT=g4[:, cc, j, dc * P:(dc + 1) * P],
                                rhs=tv[:, :, j, :],
                                start=False, stop=(i == n_mm),
                                skip_group_check=True,
                            )
                if mode == "pe_only":
                    return
                if mode == "dma_out":
                    ost = static_ost
                else:
                    ost = outp.tile([P, 512], f32, tag="ost", name="ost")
                    if cp_state[0] % 2 == 0:
                        nc.vector.tensor_scalar_max(ost, ps3, 0.0)
                    else:
                        nc.scalar.activation(ost, ps3, relu)
                    cp_state[0] += 1
                if mode == "no_dma":
                    return
                mbase = g * MG
                for dc in range(NC2):
                    dst = Od[mbase:mbase + MG, dc * P:(dc + 1) * P, :]
                    deng = nc.sync if dc == 0 else nc.scalar
                    deng.dma_start(
                        out=dst.rearrange("m d h -> d m h"),
                        in_=ost[:, dc * N:(dc + 1) * N],
                    )

            LOOKAHEAD = 5
            for _rep in range(reps):
                phase1()
                for g in range(LOOKAHEAD):
                    trans(g)
                pending = []
                for g in range(NG):
                    if g + LOOKAHEAD < NG:
                        trans(g + LOOKAHEAD)
                    if len(pending) >= 2:
                        go, tss = pending.pop(0)
                        ps3 = ps3p.tile([P, 512], f32, tag="ps3",
                                        name="ps3t")
                        p3_start(ps3)
                        final(go, tss, ps3)
                    pending.append((g, tbuild(g)))
                for go, tss in pending:
                    ps3 = ps3p.tile([P, 512], f32, tag="ps3", name="ps3t")
                    p3_start(ps3)
                    final(go, tss, ps3)

    if split_waits:
        _split_multi_waits(nc)
    return nc


def _split_multi_waits(nc):
    """This walrus build accepts at most one semaphore wait per
    instruction; Tile emits up to ~2-4.  Hoist extra waits onto NoOp
    instructions inserted just before, on the same engine."""
    import concourse.mybir as mybir

    n_split = 0
    for fn in nc.m.functions:
        for bb in fn.blocks:
            insts = bb.instructions
            new = []
            for inst in insts:
                si = inst.sync_info
                waits = list(si.on_wait) if si is not None else []
                if len(waits) > 1:
                    for w in waits[:-1]:
                        nop = mybir.InstNoOp(
                            name=nc.get_next_instruction_name(), ins=[], outs=[]
                        )
                        nop.engine = inst.engine
                        nop.sync_info = mybir.SyncInfo(
                            on_update=[], on_wait=[w]
                        )
                        new.append(nop)
                        n_split += 1
                    si.on_wait = [waits[-1]]
                new.append(inst)
            if n_split:
                bb.instructions = new
    return n_split


def _get_nc():
    if "nc" not in _CACHE:
        _CACHE["nc"] = _build_nc()
    return _CACHE["nc"]


def _prep(G, W, b):
    # T-build weights: rows klp = k*32+l (padded 96->128), cols (j, h).
    Wt = np.zeros((P, K * H), dtype=np.float32)
    for k in range(K):
        for j in range(K):
            blk = W[k * (K * L) + j * L:k * (K * L) + (j + 1) * L, :]  # [L,H]
            Wt[k * L:(k + 1) * L, j * H:(j + 1) * H] = blk
    Br = np.tile(b, 8)[None, :].astype(np.float32)
    return np.ascontiguousarray(Wt), Br


def _make_in_maps(X, G, W, b):
    import ml_dtypes

    X = np.ascontiguousarray(X, dtype=np.float32)
    G = np.ascontiguousarray(G, dtype=np.float32)
    W = np.ascontiguousarray(W, dtype=np.float32)
    b = np.ascontiguousarray(b, dtype=np.float32)
    Wt, Br = _prep(G, W, b)
    Xb = X.astype(ml_dtypes.bfloat16)
    Gb = G.astype(ml_dtypes.bfloat16)
    Wtb = Wt.astype(ml_dtypes.bfloat16)
    Brb = Br.astype(ml_dtypes.bfloat16)
    return [
        {"X": Xb[i], "GB": Gb, "WT": Wtb, "BR": Brb} for i in range(B)
    ]


def kernel(X, G, W, b):
    from concourse.bass_utils import run_bass_kernel_spmd

    nc = _get_nc()
    in_maps = _make_in_maps(X, G, W, b)
    res = run_bass_kernel_spmd(nc, in_maps, list(range(B)))
    out = np.stack([res.results[i]["OUT"] for i in range(B)], axis=0)
    return out


# revision 36
# speedup vs baseline: 1.1684x; 1.1684x over previous
"""BDGCN (dual-diffusion graph conv) Trainium2 kernel.

Math (per batch b):
  m1[k,m,c,l] = sum_n X[n,c,l] G[k,n,m]
  m2[m,d,k,j,l] = sum_c m1[k,m,c,l] G[j,c,d]
  out[m,d,h] = relu(sum_{k,j,l} m2[m,d,k,j,l] W[k*96+j*32+l, h] + b[h])

Sharding: data-parallel over batch; B=8 -> one batch per NeuronCore,
G/W/b replicated. No collectives.

Per-core pipeline, all-bf16 operands with fp32 PSUM accumulation:
  Phase 1 (contract n): stationary = X[n128, c128 @ l] bf16,
           moving = G (k-paired) [n128, (k2,m256)=512] -> psum
           [c128, (k,m)], accum over 2 n-chunks; one DVE/Pool/Act copy
           per (cc,l) into m1[cc] bf16, free layout (k,g,l,r), m=4g+r.
  Phase 2 (contract c): stationary = m1[cc][k,g] (128 cols = (l,r)),
           moving = G (j-paired) [c128, (j2,d256)] -> psum
           [(l,r)128, (j,d)], accum over 2 c-chunks -> m2 SBUF bf16
           tiles [128, 768] per (g,k).
  Phase 3 (contract (k,j,l)): stationary = m2[g,k][:, j,dc-slice],
           moving = block-diagonal W [(l,r)128, (r,h)256] -> psum
           [d128, (m4,h)256], accum over 9 (k,j). Then +bias (DVE
           scalar_tensor_tensor), relu (Pool), DMA out to [m, d, h].
  Phase 3 of group g is emitted after phase 2 of group g+1 so the PE
  never waits on the psum->SBUF copies.

Walrus-build workarounds baked in: Tile's exit drain is split into
single-wait drains (_patch_tile_drain) and any instruction carrying >1
semaphore wait gets extra waits hoisted onto NoOps (_split_multi_waits).
"""

import numpy as np

B, N, L, K, H = 8, 256, 32, 3, 64
P = 128  # partitions

_CACHE = {}


def _patch_tile_drain():
    """This container's walrus build rejects instructions carrying more
    than one semaphore wait; Tile's exit emits one drain with N waits.
    Split it into N single-wait drains."""
    import concourse.mybir as mybir
    import concourse.tile as tile

    if getattr(tile.TileContext, "_drain_split_patched", False):
        return

    def patched(self, tick_clock, wait_clock):
        from concourse.vector_clock import ScopedClock

        nc = self.nc
        probe = nc.sync.drain()
        wait_clock.add_sem_waits(
            probe.ins, ScopedClock({None: tick_clock.global_clock})
        )
        si = probe.ins.sync_info
        waits = list(si.on_wait) if si is not None else []
        if len(waits) > 1:
            si.on_wait = [waits[0]]
            for w in waits[1:]:
                d = nc.sync.drain()
                d.ins.sync_info = mybir.SyncInfo(on_update=[], on_wait=[w])
        nc.all_engine_barrier()
        assert self.sems is not None
        popped = nc._tile_sem_poison_stack.pop()
        assert popped is self._sem_poison
        nc.clear_and_free_semaphores(list(self.sems.allocated().values()))
        nc.all_engine_barrier()

    tile.TileContext._drain_and_barrier = patched
    tile.TileContext._drain_split_patched = True


def _build_nc(reps=1, mode="full", split_waits=True):
    import concourse.bass as bass
    import concourse.mybir as mybir
    import concourse.tile as tile

    _patch_tile_drain()

    f32 = mybir.dt.float32
    bf16 = mybir.dt.bfloat16
    nc = bass.Bass("TRN2", target_bir_lowering=False, debug=False)

    Xd = nc.dram_tensor("X", [N, N, L], bf16, kind="ExternalInput")
    GBd = nc.dram_tensor("GB", [K, N, N], bf16, kind="ExternalInput")
    Wr = nc.dram_tensor("WR", [K * K, P, 4 * H], bf16, kind="ExternalInput")
    Bd = nc.dram_tensor("BR", [1, 8 * H], bf16, kind="ExternalInput")
    Od = nc.dram_tensor("OUT", [N, N, H], f32, kind="ExternalOutput")

    NC2 = N // P  # 2 chunks of 128 along n / c / d
    MG = 4       # m's per group
    NG = N // MG  # 64 groups over all m

    cp = mybir.ActivationFunctionType.Copy
    relu = mybir.ActivationFunctionType.Relu

    with tile.TileContext(nc) as tc:
        with (
            tc.tile_pool(name="big", bufs=1) as big,
            tc.tile_pool(name="m2p", bufs=12) as m2p,
            tc.tile_pool(name="outp", bufs=4) as outp,
            tc.tile_pool(name="tp", bufs=6, space="PSUM") as tpp,
            tc.tile_pool(name="ps3", bufs=2, space="PSUM") as ps3p,
        ):
            # ---- resident loads ----
            xsb = big.tile([P, NC2 * N * L], bf16, tag="xsb")
            x4 = xsb.rearrange("p (b c l) -> p b c l", b=NC2, c=N)
            nc.sync.dma_start(
                out=x4, in_=Xd[:, :, :].rearrange("(b p) c l -> p b c l", p=P)
            )
            gsb = big.tile([P, NC2 * K * N], bf16, tag="gsb")
            g4 = gsb.rearrange("p (b k m) -> p b k m", b=NC2, k=K)
            for k in range(K):
                nc.sync.dma_start(
                    out=g4[:, :, k, :],
                    in_=GBd[k, :, :].rearrange("(b p) m -> p b m", p=P),
                )
            wsb = big.tile([P, K * K * MG * H], bf16, tag="wsb")
            w3 = wsb.rearrange("p (q c) -> p q c", q=K * K)
            nc.sync.dma_start(
                out=w3, in_=Wr[:, :, :].rearrange("q p c -> p q c")
            )
            brow = big.tile([1, 8 * H], bf16, tag="brow")
            nc.sync.dma_start(out=brow, in_=Bd[:, :])
            ones = big.tile([1, P], bf16, tag="ones")
            nc.gpsimd.memset(ones[:, :], 1.0)

            m1 = {}
            for cc in range(NC2):
                t = big.tile(
                    [P, K * NG * L * MG], bf16,
                    tag=f"m1_{cc}", name=f"m1_{cc}",
                )
                if mode == "pe_only":
                    nc.gpsimd.memset(t[:, :], 0.25)
                m1[cc] = t.rearrange(
                    "p (k g l r) -> p k g l r", k=K, g=NG, l=L
                )

            static_m2 = None
            if mode == "pe_only":
                static_m2 = big.tile([P, 512], bf16, tag="sm2", name="sm2")
                nc.gpsimd.memset(static_m2[:, :], 0.25)

            cp_state = [0]

            def copy_on(out, in_):
                # psum -> SBUF; only DVE and Act can read PSUM
                if mode == "pe_only":
                    return
                e = cp_state[0] % 2
                cp_state[0] += 1
                if e == 0:
                    nc.vector.tensor_copy(out, in_)
                else:
                    nc.scalar.activation(out, in_, cp)

            def phase1():
                for cc in range(NC2):
                    bpend = None
                    for l in range(L):
                        pa = tpp.tile([P, 512], f32, tag="tp", name="pa")
                        if bpend is None:
                            bpend = tpp.tile([P, 512], f32, tag="tp", name="pb")
                            boff = 0
                        else:
                            boff = N
                        for nchk in range(NC2):
                            st = x4[:, nchk, cc * P:(cc + 1) * P, l]
                            nc.tensor.matmul(
                                pa, lhsT=st, rhs=g4[:, nchk, 0:2, :],
                                start=(nchk == 0), stop=(nchk == 1),
                                skip_group_check=True,
                            )
                            nc.tensor.matmul(
                                bpend[:, boff:boff + N], lhsT=st,
                                rhs=g4[:, nchk, 2, :],
                                start=(nchk == 0), stop=(nchk == 1),
                                skip_group_check=True,
                            )
                        # psum cols (k,m), m = 4g+r -> m1[cc][k, g, l, r]
                        copy_on(
                            m1[cc][:, 0:2, :, l, :],
                            pa.rearrange("p (k g r) -> p k g r", k=2, g=NG),
                        )
                        if boff == N:
                            copy_on(
                                m1[cc][:, 2, :, l - 1:l + 1, :],
                                bpend.rearrange(
                                    "p (h g r) -> p g h r", h=2, g=NG
                                ),
                            )
                            bpend = None

            # ---- phases 2 + 3, interleaved at (k,j) granularity ----
            st2 = {}

            def p2_reset():
                st2.update(pend=None, m2=[], nq=0)

            def p2_unit(g, q):
                k, j = divmod(q, K)
                if st2["pend"] is None:
                    st2["pend"] = tpp.tile([P, 512], f32, tag="tp", name="t2")
                    off = 0
                else:
                    off = N
                T = st2["pend"]
                for cc in range(NC2):
                    nc.tensor.matmul(
                        T[:, off:off + N],
                        lhsT=m1[cc][:, k, g, :, :],  # 128 cols = (l, r)
                        rhs=g4[:, cc, j, :],
                        start=(cc == 0), stop=(cc == 1),
                        skip_group_check=True,
                    )
                if off == N:
                    if mode == "pe_only":
                        st2["m2"].append(static_m2)
                        st2["m2"].append(static_m2)
                    else:
                        t = m2p.tile([P, 512], bf16, tag="m2", name="m2t")
                        copy_on(t, T)
                        st2["m2"].append(t)
                        st2["m2"].append(t)
                    st2["pend"] = None

            def p3_start(ps3):
                # rank-1 bias seed over both dc halves: ps3 = ones^T @ brow.
                # start=True zeroes the whole psum bank once; every later
                # (k,j) matmul accumulates with start=False.
                nc.tensor.matmul(
                    ps3, lhsT=ones, rhs=brow,
                    start=True, stop=False, skip_group_check=True,
                )

            def p3_unit(g, q, ps3):
                qg = g * K * K + q
                t = st2["m2"][qg]
                base = (qg % 2) * N
                for dc in range(NC2):
                    nc.tensor.matmul(
                        ps3[:, dc * N:dc * N + MG * H],
                        lhsT=t[:, base + dc * P:base + (dc + 1) * P],
                        rhs=w3[:, q, :],
                        start=False, stop=(q == K * K - 1 and dc == NC2 - 1),
                        skip_group_check=True,
                    )

            def p3_finish(g, ps3):
                if mode == "pe_only":
                    return
                ost = outp.tile([P, 512], f32, tag="ost", name="ost")
                if cp_state[0] % 2 == 0:
                    nc.vector.tensor_scalar_max(ost, ps3, 0.0)
                else:
                    nc.scalar.activation(ost, ps3, relu)
                cp_state[0] += 1
                if mode == "no_dma":
                    return
                mbase = g * MG
                for dc in range(NC2):
                    dst = Od[mbase:mbase + MG, dc * P:(dc + 1) * P, :]
                    nc.sync.dma_start(
                        out=dst.rearrange("m d h -> d m h"),
                        in_=ost[:, dc * N:(dc + 1) * N],
                    )

            for _rep in range(reps):
                phase1()
                p2_reset()
                ps3 = None
                for g in range(NG):
                    if g > 0:
                        ps3 = ps3p.tile([P, 512], f32, tag="ps3", name="ps3t")
                        p3_start(ps3)
                    for q in range(K * K):
                        p2_unit(g, q)
                        if g > 0:
                            p3_unit(g - 1, q, ps3)
                    if g > 0:
                        p3_finish(g - 1, ps3)
                ps3 = ps3p.tile([P, 512], f32, tag="ps3", name="ps3t")
                p3_start(ps3)
                for q in range(K * K):
                    p3_unit(NG - 1, q, ps3)
                p3_finish(NG - 1, ps3)

    if split_waits:
        _split_multi_waits(nc)
    return nc


def _split_multi_waits(nc):
    """This walrus build accepts at most one semaphore wait per
    instruction; Tile emits up to ~2-4.  Hoist extra waits onto NoOp
    instructions inserted just before, on the same engine."""
    import concourse.mybir as mybir

    n_split = 0
    for fn in nc.m.functions:
        for bb in fn.blocks:
            insts = bb.instructions
            new = []
            for inst in insts:
                si = inst.sync_info
                waits = list(si.on_wait) if si is not None else []
                if len(waits) > 1:
                    for w in waits[:-1]:
                        nop = mybir.InstNoOp(
                            name=nc.get_next_instruction_name(), ins=[], outs=[]
                        )
                        nop.engine = inst.engine
                        nop.sync_info = mybir.SyncInfo(
                            on_update=[], on_wait=[w]
                        )
                        new.append(nop)
                        n_split += 1
                    si.on_wait = [waits[-1]]
                new.append(inst)
            if n_split:
                bb.instructions = new
    return n_split


def _get_nc():
    if "nc" not in _CACHE:
        _CACHE["nc"] = _build_nc()
    return _CACHE["nc"]


def _prep(G, W, b):
    # Block-diagonal W for phase 3: rows indexed (l, r) with r = m-within-
    # group, cols (r'', h); nonzero only when r == r''.
    MG = 4
    Wbd = np.zeros((K * K, P, MG * H), dtype=np.float32)
    for k in range(K):
        for j in range(K):
            blk = W[k * (K * L) + j * L:k * (K * L) + (j + 1) * L, :]  # [L, H]
            for l in range(L):
                for r in range(MG):
                    Wbd[k * K + j, l * MG + r, r * H:(r + 1) * H] = blk[l]
    Br = np.tile(b, 8)[None, :].astype(np.float32)
    return np.ascontiguousarray(Wbd), Br


def _make_in_maps(X, G, W, b):
    import ml_dtypes

    X = np.ascontiguousarray(X, dtype=np.float32)
    G = np.ascontiguousarray(G, dtype=np.float32)
    W = np.ascontiguousarray(W, dtype=np.float32)
    b = np.ascontiguousarray(b, dtype=np.float32)
    Wr, Br = _prep(G, W, b)
    Xb = X.astype(ml_dtypes.bfloat16)
    Gb = G.astype(ml_dtypes.bfloat16)
    Wrb = Wr.astype(ml_dtypes.bfloat16)
    Brb = Br.astype(ml_dtypes.bfloat16)
    return [
        {"X": Xb[i], "GB": Gb, "WR": Wrb, "BR": Brb} for i in range(B)
    ]


def kernel(X, G, W, b):
    from concourse.bass_utils import run_bass_kernel_spmd

    nc = _get_nc()
    in_maps = _make_in_maps(X, G, W, b)
    res = run_bass_kernel_spmd(nc, in_maps, list(range(B)))
    out = np.stack([res.results[i]["OUT"] for i in range(B)], axis=0)
    return out
